# revision 6
# baseline (speedup 1.0000x reference)
"""GPTQ 4-bit quantized linear (column-parallel over 8 NeuronCores), v4.

y = x @ dequant(qweight, scales, zeros).T + bias with byte-packed 4-bit pairs.

Core trick: fp8e4m3 bit patterns 0..15 decode to exactly b * 2^-9 (denormals
plus the first normal binade are uniformly spaced), so nibbles extracted with
cheap fused DVE bitwise ops (and / shr+and on u16 lanes) can be BITCAST to
fp8e4 and fed to the PE against an fp16 x (mixed-dtype matmul): no cast
instructions and no fp8 x-split. The 2^9 is folded into the host scales.

Layout per core (out_features 11008 -> 8 x 1376):
  * k-tiles 0-11 ship packed (0.5 B/weight = the DMA floor); k12-k15 ship
    as host-unpacked raw nibble bytes (same fp8-bitcast encoding, 2x bytes)
    so the tail of the stream needs no decode work at all.
  * D[o, t, c, b] per-group dots accumulate in PSUM; fix pieces [8,2,2,2,2]
    k-tiles: ACT copies PSUM to SBUF f16 [p,t,b,c] (transposing copy), the
    sfix multiply runs in DVE 2x mode (b-broadcast, c innermost), a halving
    TT-tree reduces over c, and two alternating f16 accumulators (uA/uB)
    keep the serial chain short. Piece 2 and the uB fold run on the
    otherwise-idle Pool engine (from SBUF; GPSIMD cannot touch PSUM, and
    TensorScalarPtr masks are DVE-only on real hardware); the last piece
    multiplies straight from PSUM on DVE to skip the ACT hop on the tail.
  * Output: yt [128, 88] f16, one straight-copy DMA; host de-tiles.
"""

import numpy as np

import concourse.bacc as bacc
import concourse.mybir as mybir
import concourse.tile as tile
from concourse.bass_utils import run_bass_kernel_spmd

dt = mybir.dt
Alu = mybir.AluOpType

B = 8
I = 4096
O = 11008
NCORES = 8
OSH = O // NCORES            # 1376
OT = 11
OPW = OSH                    # 1376
NG = 32
NJ = 16                      # k-tiles (2048 / 128)

NPK = 12                     # packed k-tiles; k12-k15 ship as f8 bytes
PIECES = [8, 2, 2, 2, 2]     # k-tiles per fix piece (2 groups per tile)
# hi-mask engine per packed pair: "v" = DVE, "p" = Pool (gpsimd)
HI_ROUTES = ["v", "v", "v", "v", "v", "v"]

_piece_of = []
_slot_of = []
for _pi, _n in enumerate(PIECES):
    for _j in range(_n):
        _piece_of.append(_pi)
        _slot_of.append(2 * _j)

_nc_cache = None


def _build_nc():
    nc = bacc.Bacc("TRN2", target_bir_lowering=False)

    wq_d = {k0: nc.dram_tensor(f"wq{k0}", [2, 128, OPW], dt.uint8,
                               kind="ExternalInput") for k0 in range(0, NPK, 2)}
    f8_d = {"f8t12": nc.dram_tensor("f8t12", [128, 4 * OPW], dt.uint8,
                                    kind="ExternalInput"),
            "f8t14": nc.dram_tensor("f8t14", [128, 2 * OPW], dt.uint8,
                                    kind="ExternalInput"),
            "f8t15": nc.dram_tensor("f8t15", [128, 2 * OPW], dt.uint8,
                                    kind="ExternalInput")}  # k14/k15 split
    cblob_d = nc.dram_tensor("cblob", [128, 1392], dt.uint8, kind="ExternalInput")
    out = nc.dram_tensor("out", [128, OT * B], dt.float16, kind="ExternalOutput")

    with tile.TileContext(nc) as tc:
        with (
            tc.tile_pool(name="const", bufs=1) as constp,
            tc.tile_pool(name="wqp", bufs=4) as wqp,
            tc.tile_pool(name="nibp", bufs=6) as nibp,
            tc.tile_pool(name="fixp", bufs=1) as fixp,
            tc.tile_pool(name="dpsp", bufs=1, space="PSUM") as dpsp,
        ):
            cblob = constp.tile([128, 1392], dt.uint8)
            wq_sb = {}

            def wq_dma(k0):
                wq_sb[k0] = wqp.tile([128, 2, OPW], dt.uint8, name=f"wq{k0}")
                nc.sync.dma_start(wq_sb[k0][:],
                                  wq_d[k0][:].rearrange("n p o -> p n o"))

            # stream: wq0, cblob, wq2..wq10, f8 tiles last
            wq_dma(0)
            nc.sync.dma_start(cblob[:], cblob_d[:])
            xt_sb = cblob[:, 0:512].bitcast(dt.float16)          # [128, 32*8]
            sfix_sb = cblob[:, 512:1216].bitcast(dt.float16)     # [128, 11*32]
            corrb_sb = cblob[:, 1216:1392].bitcast(dt.float16)   # [128, 11*8]
            for k0 in (2, 4, 6, 8, 10):
                wq_dma(k0)
            f8_sb = {}
            f8_sb["f8t12"] = constp.tile([128, 4 * OPW], dt.uint8, name="f8t12")
            nc.sync.dma_start(f8_sb["f8t12"][:], f8_d["f8t12"][:])
            for nm in ("f8t14", "f8t15"):
                f8_sb[nm] = constp.tile([128, 2 * OPW], dt.uint8, name=nm)
                nc.sync.dma_start(f8_sb[nm][:], f8_d[nm][:])

            # PSUM: piece tiles (3+2+1+1+1 banks)
            d_ps = [dpsp.tile([128, OT, 2 * n, 8], dt.float32, name=f"d{i}",
                              tag=f"d{i}") for i, n in enumerate(PIECES)]

            def decode_pair(k0):
                src16 = wq_sb[k0][:].rearrange("p n o -> p (n o)").bitcast(dt.uint16)
                ehi = nc.gpsimd if HI_ROUTES[k0 // 2] == "p" else nc.vector
                lo8 = nibp.tile([128, 2, OPW], dt.uint8, tag="lo8", name=f"lo8_{k0}")
                nc.vector.tensor_scalar(
                    lo8[:].rearrange("p n o -> p (n o)").bitcast(dt.uint16),
                    src16, 0x0F0F, None, op0=Alu.bitwise_and)
                hi8 = nibp.tile([128, 2, OPW], dt.uint8, tag="hi8", name=f"hi8_{k0}")
                ehi.tensor_scalar(
                    hi8[:].rearrange("p n o -> p (n o)").bitcast(dt.uint16),
                    src16, 4, 0x0F0F,
                    op0=Alu.logical_shift_right, op1=Alu.bitwise_and)
                return lo8, hi8

            def mms(k, lo_ap, hi_ap):
                # lo_ap/hi_ap: [128, OPW] uint8 views holding nibble values
                dtile = d_ps[_piece_of[k]]
                clo = _slot_of[k]
                glo, ghi = k, k + 16
                for t in range(OT):
                    w = 128 if t < OT - 1 else OPW - 128 * (OT - 1)
                    dv = dtile[: w, t]
                    nc.tensor.matmul(dv[:, clo],
                                     lo_ap[:, t * 128:t * 128 + w].bitcast(dt.float8e4),
                                     xt_sb[:, glo * B:(glo + 1) * B],
                                     start=True, stop=True)
                    nc.tensor.matmul(dv[:, clo + 1],
                                     hi_ap[:, t * 128:t * 128 + w].bitcast(dt.float8e4),
                                     xt_sb[:, ghi * B:(ghi + 1) * B],
                                     start=True, stop=True)

            uA = fixp.tile([128, OT * B], dt.float16, name="uA")
            uB = fixp.tile([128, OT * B], dt.float16, name="uB")
            yt = fixp.tile([128, OT * B], dt.float16, name="yt")

            def fix_piece(pi):
                veng = nc.gpsimd if pi == 2 else nc.vector
                ncols = 2 * PIECES[pi]
                c0 = sum(2 * n for n in PIECES[:pi])
                dv = d_ps[pi][:]
                sf = sfix_sb.rearrange("p (t c) -> p t c", c=NG)[
                    :, :, c0:c0 + ncols]
                tmp = fixp.tile([128, OT, B, ncols], dt.float16,
                                tag=f"tmp{pi}", name=f"tmp{pi}")
                if pi == len(PIECES) - 1:
                    # last piece: multiply straight from PSUM on DVE (1x but
                    # small; skips the ACT hop on the critical tail). GPSIMD
                    # cannot read PSUM, so only DVE pieces may do this.
                    nc.vector.tensor_tensor(
                        tmp[:], dv.transpose([0, 1, 3, 2]),
                        sf.unsqueeze(2).broadcast_to([128, OT, B, ncols]),
                        Alu.mult)
                else:
                    # ACT: PSUM f32 [p,t,c,b] -> SBUF f16 [p,t,b,c]
                    cp = fixp.tile([128, OT, B, ncols], dt.float16,
                                   tag=f"cp{pi}", name=f"cp{pi}")
                    nc.scalar.copy(cp[:].transpose([0, 1, 3, 2]), dv)
                    veng.tensor_tensor(
                        tmp[:], cp[:],
                        sf.unsqueeze(2).broadcast_to([128, OT, B, ncols]),
                        Alu.mult)
                # halve over c down to 2 columns (2x TT tree), then fold the
                # piece into one of two alternating accumulator chains
                with nc.allow_low_precision(reason="f16 partial sums"):
                    w = ncols
                    while w > 2:
                        h = w // 2
                        veng.tensor_tensor(
                            tmp[:, :, :, 0:h], tmp[:, :, :, 0:h],
                            tmp[:, :, :, h:w], Alu.add)
                        w = h
                    red = fixp.tile([128, OT, B], dt.float16, tag=f"red{pi}",
                                    name=f"red{pi}")
                    veng.tensor_tensor(red[:], tmp[:, :, :, 0],
                                       tmp[:, :, :, 1], Alu.add)
                    rv = red[:].rearrange("p t b -> p (t b)")
                    npc = len(PIECES)
                    if pi == 0:
                        nc.vector.tensor_tensor(uA[:], corrb_sb, rv, Alu.add)
                    elif pi == 1:
                        nc.vector.tensor_copy(uB[:], rv)
                    elif pi == npc - 1:
                        # last piece: yt = uAB + red (uAB folded in at npc-2)
                        nc.vector.tensor_tensor(yt[:], uA[:], rv, Alu.add)
                    elif pi == npc - 2:
                        # fold uB on the idle Pool engine, off the DVE tail
                        nc.gpsimd.tensor_tensor(uB[:], uB[:], rv, Alu.add)
                        nc.vector.tensor_tensor(uA[:], uA[:], uB[:], Alu.add)
                    else:
                        eng = uA if pi % 2 == 0 else uB
                        nc.vector.tensor_tensor(eng[:], eng[:], rv, Alu.add)

            npieces = len(PIECES)
            fix_after = {}
            acc = 0
            for pi, n in enumerate(PIECES):
                acc += n
                fix_after[acc - 1] = pi

            for k0 in range(0, NPK, 2):
                lo8, hi8 = decode_pair(k0)
                for j in (0, 1):
                    k = k0 + j
                    mms(k, lo8[:, j], hi8[:, j])
                    if k in fix_after:
                        fix_piece(fix_after[k])
            # f8 tiles k12-k15 (nibble bytes: [lo hi] per tile)
            for j, k in enumerate((12, 13)):
                mms(k, f8_sb["f8t12"][:, (2 * j) * OPW:(2 * j + 1) * OPW],
                    f8_sb["f8t12"][:, (2 * j + 1) * OPW:(2 * j + 2) * OPW])
                if k in fix_after:
                    fix_piece(fix_after[k])
            for k in (14, 15):
                t = f8_sb[f"f8t{k}"]
                mms(k, t[:, 0:OPW], t[:, OPW:2 * OPW])
                if k in fix_after:
                    fix_piece(fix_after[k])

            nc.sync.dma_start(out[:], yt[:])

    nc.compile()
    return nc


def _get_nc():
    global _nc_cache
    if _nc_cache is None:
        _nc_cache = _build_nc()
    return _nc_cache


def _prep_inputs(x, qweight, scales, zeros, bias):
    x = np.asarray(x)
    qweight = np.asarray(qweight)
    scales = np.asarray(scales)
    zeros = np.asarray(zeros)
    bias = np.asarray(bias)

    qb = qweight.astype(np.uint8)
    nib = np.empty((O, I), np.uint8)
    nib[:, 0::2] = qb & 15
    nib[:, 1::2] = qb >> 4
    nibT = np.ascontiguousarray(nib.T)                    # (4096, 11008)
    wq_packed = nibT[: I // 2] | (nibT[I // 2:] << 4)     # (2048, 11008)

    xt_host = np.ascontiguousarray(
        x.T.reshape(NG, 128, B).transpose(1, 0, 2).reshape(128, NG * B)
    ).astype(np.float16)

    Xg = x.astype(np.float64).T.reshape(NG, 128, B).sum(axis=1)   # (32, 8)

    piece_base = []
    acc = 0
    for n in PIECES:
        piece_base.append(acc)
        acc += 2 * n
    col_lo = {}
    for k in range(NJ):
        col_lo[k] = piece_base[_piece_of[k]] + _slot_of[k]

    in_maps = []
    for cidx in range(NCORES):
        sl = slice(cidx * OSH, (cidx + 1) * OSH)
        s_c = scales[sl].astype(np.float64)      # (1376, 32)
        z_c = zeros[sl].astype(np.float64)
        b_c = bias[sl].astype(np.float64)
        wq_c = np.ascontiguousarray(wq_packed[:, sl])

        im = {}
        kt = wq_c.reshape(NJ, 128, OPW)
        for k0 in range(0, NPK, 2):
            im[f"wq{k0}"] = np.ascontiguousarray(kt[k0:k0 + 2])
        im["f8t12"] = np.ascontiguousarray(np.concatenate(
            [kt[12] & 15, kt[12] >> 4, kt[13] & 15, kt[13] >> 4], axis=1))
        for k in (14, 15):
            im[f"f8t{k}"] = np.ascontiguousarray(np.concatenate(
                [kt[k] & 15, kt[k] >> 4], axis=1))

        # sfix: value = s * 512 (fp8 bitcast carries 2^-9)
        sfix = np.zeros((OT * 128, NG), np.float64)
        for k in range(NJ):
            sfix[:OSH, col_lo[k]] = s_c[:, k] * 512.0
            sfix[:OSH, col_lo[k] + 1] = s_c[:, k + 16] * 512.0
        sfix_t = np.ascontiguousarray(
            sfix.reshape(OT, 128, NG).transpose(1, 0, 2).reshape(128, OT * NG)
        ).astype(np.float16)

        corr = (s_c * z_c) @ Xg                  # (1376, 8)
        corrb = np.zeros((OT * 128, B), np.float64)
        corrb[:OSH] = b_c[:, None] - corr
        corrb_t = np.ascontiguousarray(
            corrb.reshape(OT, 128, B).transpose(1, 0, 2).reshape(128, OT * B)
        ).astype(np.float16)

        cblob = np.concatenate([
            xt_host.view(np.uint8),
            sfix_t.view(np.uint8),
            corrb_t.view(np.uint8),
        ], axis=1)
        assert cblob.shape == (128, 1392), cblob.shape
        im["cblob"] = np.ascontiguousarray(cblob)
        in_maps.append(im)
    return in_maps


def _gather(results):
    outs = []
    for r in results:
        o = r["out"]                              # (128, 88) f16
        y = o.reshape(128, OT, B).transpose(1, 0, 2).reshape(OT * 128, B)
        outs.append(y[:OSH])
    y = np.concatenate(outs, axis=0)
    return np.ascontiguousarray(y.T)


def kernel(x, qweight, scales, zeros, bias, _trace=False):
    nc = _get_nc()
    in_maps = _prep_inputs(x, qweight, scales, zeros, bias)
    res = run_bass_kernel_spmd(
        nc, in_maps, core_ids=list(range(NCORES)), trace=_trace
    )
    out = _gather(res.results)
    if _trace:
        return out, res
    return out
